# revision 3
# baseline (speedup 1.0000x reference)
"""Trainium2 Bass kernel for the CodingLoss problem.

Math (B=16384, N=D=1000, label smoothing 0.1):
    similarity S[b,n] = o_b . c_n + (1-o_b) . (1-c_n)
                      = 2*M[b,n] + (D - r_b) - c_n   (M = o @ cb^T, c_n = row
    sums of code_book). The per-row constant cancels in the softmax, so with
    A[b,n] = 2*M[b,n] - c_n:
        loss_b = lse(A_b) - 0.9*A[b, l_b] - (0.1/N) * sum_n A[b,n]
        output = mean_b loss_b

Device strategy (data-parallel over batch, 8 cores x 2048 rows):
  - The device computes ONLY the lse term (the part that needs the full
    [B, N] logits). The label and uniform-sum terms are exact O(B*D)
    matvecs computed on the host in fp64.
  - Host ships x pre-transposed into matmul lhsT layout (fp16), so the
    tensor engine does zero transposes: per 128-row block it runs just the
    16 accumulating matmuls (8 K-chunks x 2 PSUM banks).
  - The -(c_n + 25) bias rides the matmul as two spare K-rows (d=1000,1001
    carry fp16 value + fp16 residual against ones-columns in x), so PSUM
    holds A - 25 directly and no vector-engine pass is needed at all.
  - ScalarE exp with fused row-sum (accum_out) is the only non-PE work per
    block; logits are in [-54, 51] so exp(A-25) never overflows fp32.
  - Each core writes ln(sum exp) per row as [128, 16]; host adds back 25,
    subtracts the label/uniform terms, and averages all 16384 rows.
"""

import numpy as np

B_FULL = 16384
D = 1000
N = 1000
DPAD = 1024  # padded contraction; d=1000,1001 are the bias rows, rest zeros
KCH = 8  # K chunks of 128
NCORES = 8
BSH = B_FULL // NCORES  # 2048 rows per core
NBLK = BSH // 128  # 16 blocks of 128 rows
N1 = 512  # psum bank boundary
SMOOTH = 0.1
W_LABEL = 1.0 - SMOOTH  # 0.9
W_UNIF = SMOOTH / N  # 1e-4
EXP_BIAS = 25.0  # exp computes exp(A - 25) to keep row sums in fp32 range

_CACHE = {}


def _build_program(repeat=1):
    """repeat>1 re-processes the same inputs N times (benchmarking only:
    device time per pass = slope between repeat counts)."""
    import concourse.bass as bass
    import concourse.tile as tile
    from concourse import bacc, mybir
    from contextlib import ExitStack

    f32 = mybir.dt.float32
    f16 = mybir.dt.float16
    Act = mybir.ActivationFunctionType

    nc = bacc.Bacc("TRN2", target_bir_lowering=False, debug=False,
                   num_devices=NCORES)

    # xh[128*i + p, 128*k + j] = xpad[128*i + j, 128*k + p]: block i's lhsT
    # chunks live at rows [128i, 128i+128), chunk k at cols [128k, 128k+128)
    xh = nc.dram_tensor("xh", [BSH, DPAD], f16, kind="ExternalInput").ap()
    # rh[p, N*k + n] = 2*cb[n, 128k + p]; chunk 7 rows 104/105 hold the
    # -(c_n + 25) bias split into fp16 value + residual
    rh = nc.dram_tensor("rh", [128, KCH * N], f16, kind="ExternalInput").ap()
    lse = nc.dram_tensor("lse", [128, NBLK], f32, kind="ExternalOutput").ap()

    with tile.TileContext(nc) as tc, ExitStack() as ctx:
        rpool = ctx.enter_context(tc.tile_pool(name="rhs", bufs=1))
        xpool = ctx.enter_context(tc.tile_pool(name="x", bufs=3))
        epool = ctx.enter_context(tc.tile_pool(name="e", bufs=2))
        stat = ctx.enter_context(tc.tile_pool(name="stats", bufs=1))
        psA = ctx.enter_context(tc.tile_pool(name="psA", bufs=2, space="PSUM"))

        # chunked R load: block 0's matmuls start after chunk 0 lands (~1.5us)
        # instead of after the full 2MB; the PE clock ramps during the stall
        R = [rpool.tile([128, N], f16, tag=f"R{k}", name=f"R{k}")
             for k in range(KCH)]
        nc.sync.dma_start(R[0][:], rh[:, 0:N])
        S0 = stat.tile([128, NBLK], f32)
        S1 = stat.tile([128, NBLK], f32)
        first_x = xpool.tile([128, DPAD], f16, tag="xt")
        nc.sync.dma_start(first_x[:], xh[0:128, :])
        for k in range(1, KCH):
            nc.sync.dma_start(R[k][:], rh[:, k * N:(k + 1) * N])

        for it in range(NBLK * repeat):
            i = it % NBLK
            if it == 0:
                xt = first_x
            else:
                xt = xpool.tile([128, DPAD], f16, tag="xt")
                nc.sync.dma_start(xt[:], xh[i * 128:(i + 1) * 128, :])

            # logits A - 25 accumulate into one 2-bank PSUM tile; bank 0's
            # group completes first so its exp overlaps bank 1's matmuls
            pA = psA.tile([128, 1024], f32, tag="pA")
            for k in range(KCH):
                nc.tensor.matmul(pA[:, 0:N1], xt[:, k * 128:(k + 1) * 128],
                                 R[k][:, 0:N1],
                                 start=(k == 0), stop=(k == KCH - 1))
            e0 = epool.tile([128, N1], f32, tag="e0")
            nc.scalar.activation(e0[:], pA[:, 0:N1], Act.Exp,
                                 accum_out=S0[:, i:i + 1])
            for k in range(KCH):
                nc.tensor.matmul(pA[:, N1:N], xt[:, k * 128:(k + 1) * 128],
                                 R[k][:, N1:N],
                                 start=(k == 0), stop=(k == KCH - 1))
            e1 = epool.tile([128, N - N1], f32, tag="e1")
            nc.scalar.activation(e1[:], pA[:, N1:N], Act.Exp,
                                 accum_out=S1[:, i:i + 1])

        S = stat.tile([128, NBLK], f32)
        nc.vector.tensor_tensor(S[:], S0[:], S1[:], Alu.add)
        out_t = stat.tile([128, NBLK], f32)
        nc.scalar.activation(out_t[:], S[:], Act.Ln)
        nc.sync.dma_start(lse, out_t[:])

    nc.compile()  # bacc passes: wait legalization (<=1 sync wait/instr), DCE
    return nc


def _get_nc(repeat=1):
    key = ("nc", repeat)
    if key not in _CACHE:
        _CACHE[key] = _build_program(repeat)
    return _CACHE[key]


def _prep_inputs(inputs, labels, code_book):
    """Host-side shard/pack prep. Returns per-core input maps."""
    x = np.asarray(inputs, dtype=np.float32)
    cb = np.asarray(code_book, dtype=np.float32)

    cb64 = cb.astype(np.float64)
    c = cb64.sum(1)  # [N] row sums
    t = -(c + EXP_BIAS)
    s1 = t.astype(np.float16)
    s2 = (t - s1.astype(np.float64)).astype(np.float16)

    rh = np.zeros((128, KCH * N), dtype=np.float16)
    cbT2 = np.ascontiguousarray((2.0 * cb).T.astype(np.float16))  # [D, N]
    for k in range(KCH):
        d0 = 128 * k
        dw = min(128, D - d0)
        rh[:dw, k * N:(k + 1) * N] = cbT2[d0:d0 + dw, :]
    rh[104, 7 * N:8 * N] = s1  # d = 1000
    rh[105, 7 * N:8 * N] = s2  # d = 1001

    xpad = np.zeros((B_FULL, DPAD), dtype=np.float16)
    xpad[:, :D] = x.astype(np.float16)
    xpad[:, D] = 1.0
    xpad[:, D + 1] = 1.0

    in_maps = []
    for ci in range(NCORES):
        xc = xpad[ci * BSH:(ci + 1) * BSH]
        xhc = np.ascontiguousarray(
            xc.reshape(NBLK, 128, KCH, 128).transpose(0, 3, 2, 1)
            .reshape(BSH, DPAD))
        in_maps.append({"xh": xhc, "rh": rh})
    return in_maps


def _host_terms(inputs, labels, code_book):
    """Exact fp64 label + uniform-sum loss terms (per row)."""
    x64 = np.asarray(inputs).astype(np.float64)
    cb64 = np.asarray(code_book).astype(np.float64)
    lab = np.asarray(labels).astype(np.int64)
    c = cb64.sum(1)
    A_lab = 2.0 * np.einsum("bd,bd->b", x64, cb64[lab]) - c[lab]
    sumA = 2.0 * (x64 @ cb64.sum(0)) - c.sum()
    return W_LABEL * A_lab + W_UNIF * sumA


def _run(inputs, labels, code_book, trace=False):
    from concourse.bass_utils import run_bass_kernel_spmd
    nc = _get_nc()
    in_maps = _prep_inputs(inputs, labels, code_book)
    res = run_bass_kernel_spmd(nc, in_maps, list(range(NCORES)), trace=trace)
    lse_dev = np.stack([res.results[c]["lse"] for c in range(NCORES)])
    # [core, p, i] -> row b = core*2048 + i*128 + p
    lse_rows = lse_dev.transpose(0, 2, 1).reshape(-1).astype(np.float64)
    loss = (lse_rows + EXP_BIAS) - _host_terms(inputs, labels, code_book)
    return np.float32(loss.mean()), res


def kernel(inputs, labels, code_book):
    out, _ = _run(inputs, labels, code_book)
    return np.asarray(out, dtype=np.float32)
